# revision 47
# baseline (speedup 1.0000x reference)
"""Trainium2 Bass kernel for nn_DilConv: relu -> 3x3 depthwise dilated conv
(dilation=2, pad=2) -> 1x1 pointwise conv (192->192) -> BatchNorm (training
mode, global batch stats) on x[64,192,64,64] f32.

Sharding: data-parallel over batch N across 8 cores (8 images/core).
Sync-BN via an AllReduce of per-channel (sum, sumsq) of z.

v2 design (vs f32r baseline):
- bf16 matmul path end to end: f32r lowers to FP32_HIGH mode (~2 cycles/row,
  no fast weight load); bf16 streams 1 cycle/row with FWL. x is cast to bf16
  on the host, DMA'd into padded window tiles, relu'd in place on DVE.
- Depthwise loops are tap-outer over 2-slice groups so consecutive matmuls
  share lhsT (weight-load reuse) with PSUM at 4 banks double-buffered.
- Channel chunk1 (64 ch) of two images is paired on 128 partitions: one
  diagonal matmul computes both images' depthwise output (25% fewer PE
  rows). Pointwise consumes the paired y via base-64 row tiles (duplicated
  pwT_hi weights). All tile positions are row-aligned; no column tiling.
- z stays in SBUF as bf16 arenas - no DRAM roundtrip. Output is written
  bf16 and upcast on the host (rel tol 2e-2 allows it).
- dw sweeps of pair k interleave with pw steps of pair k-1 so the PE stream
  is dense while PSUM evacuations (split across ACT and DVE) keep pace.
"""

import sys

import numpy as np

sys.path.insert(0, "/opt/trn_rl_repo")

N_CORES = 8
N, C, H, W = 64, 192, 64, 64
NPER = N // N_CORES  # images per core
NPAIR = NPER // 2
K, DIL, PAD = 3, 2, 2
BN_EPS = 1e-5
HS = 8           # h rows per psum slice (8*64 = 512 = fp32 psum bank)
HHALF = 32       # rows per half-image working set
SLH = HHALF // HS  # 4 slices per half
WR = 36          # window rows per half (32 + 2*2 halo)
WC = W + 4       # 68 window cols
PIX = H * W      # 4096 pixels/image
HPIX = HHALF * W  # 2048 pixels per half
NTOT = float(N * PIX)  # global BN count


def _build(nc_mod, tile_mod, mybir):
    from contextlib import ExitStack

    f32 = mybir.dt.float32
    bf16 = mybir.dt.bfloat16
    AF = mybir.ActivationFunctionType
    OP = mybir.AluOpType

    import concourse.bacc as bacc

    nc = bacc.Bacc("TRN2", target_bir_lowering=False, debug=False,
                   num_devices=N_CORES)

    x_d = nc.dram_tensor("x", [NPER, C, H, W], bf16, kind="ExternalInput")
    dwd0_d = nc.dram_tensor("dwd0", [128, 9, 128], bf16, kind="ExternalInput")
    dwd1_d = nc.dram_tensor("dwd1p", [128, 9, 128], bf16, kind="ExternalInput")
    pwlo_d = nc.dram_tensor("pwt_lo", [128, 192], bf16, kind="ExternalInput")
    pwhi_d = nc.dram_tensor("pwt_hi", [128, 192], bf16, kind="ExternalInput")
    gb0_d = nc.dram_tensor("gb0", [2, 128], f32, kind="ExternalInput")
    gb1_d = nc.dram_tensor("gb1", [2, 64], f32, kind="ExternalInput")
    out_d = nc.dram_tensor("out", [NPER, C, H, W], bf16, kind="ExternalOutput")
    st_l = [nc.dram_tensor(f"stats_l{i}", [192, 2], f32, kind="Internal")
            for i in range(2)]
    st_g = [nc.dram_tensor(f"stats_g{i}", [N_CORES * 192, 2], f32,
                           kind="Internal", addr_space="Shared")
            for i in range(2)]

    with tile_mod.TileContext(nc) as tc, ExitStack() as ctx:
        const = ctx.enter_context(tc.tile_pool(name="const", bufs=1))
        spool = ctx.enter_context(tc.tile_pool(name="stats", bufs=1))
        zpool = ctx.enter_context(tc.tile_pool(name="zarena", bufs=1))
        p1ctx = ctx.enter_context(ExitStack())
        xrpool = p1ctx.enter_context(tc.tile_pool(name="xr", bufs=6))
        ypool = p1ctx.enter_context(tc.tile_pool(name="y", bufs=2))
        sqpool = p1ctx.enter_context(tc.tile_pool(name="sq", bufs=1))
        dwps = p1ctx.enter_context(tc.tile_pool(name="dwps", bufs=4,
                                                space="PSUM"))
        pwps = p1ctx.enter_context(tc.tile_pool(name="pwps", bufs=1,
                                                space="PSUM"))

        # ---- constants (dwd0 first: the first dw sweep needs only it) ----
        dwd0 = const.tile([128, 9, 128], bf16)
        nc.sync.dma_start(dwd0[:], dwd0_d.ap())
        dwd1 = const.tile([128, 9, 128], bf16)
        pwlo = const.tile([128, 192], bf16)
        pwhi = const.tile([128, 192], bf16)
        gb0 = const.tile([128, 2], f32)
        gb1 = const.tile([64, 2], f32)

        def load_rest_of_consts():
            nc.sync.dma_start(dwd1[:], dwd1_d.ap())
            nc.sync.dma_start(pwlo[:], pwlo_d.ap())
            nc.sync.dma_start(pwhi[:], pwhi_d.ap())
            nc.sync.dma_start(gb0[:], gb0_d.ap().rearrange("a c -> c a"))
            nc.sync.dma_start(gb1[:], gb1_d.ap().rearrange("a c -> c a"))



        # z arenas in SBUF (bf16)
        zar0 = zpool.tile([128, NPER * PIX], bf16, name="zar0")
        zar1 = zpool.tile([64, NPER * PIX], bf16, name="zar1")

        # stats arenas: one column per pw evac instruction
        sum0 = spool.tile([128, NPER * 8], f32, name="sum0")
        sq0 = spool.tile([128, NPER * 8], f32, name="sq0")
        sum1 = spool.tile([64, NPER * 8], f32, name="sum1")
        sq1 = spool.tile([64, NPER * 8], f32, name="sq1")

        # ---- phase 1 ----
        def xr_dma(src_slices, h):
            """Zero borders + DMA x into a window tile [128, WR, WC] bf16.
            The relu pass is emitted separately (xr_relu) so PSUM evacs
            queued on DVE are not head-of-line blocked behind the x DMA."""
            t = xrpool.tile([128, WR, WC], bf16, tag="xr")
            nc.vector.memset(t[:, :, 0:2], 0.0)
            nc.vector.memset(t[:, :, W + 2:W + 4], 0.0)
            if h == 0:
                nc.vector.memset(t[:, 0:2, 2:W + 2], 0.0)
                rlo, rhi, r0 = 0, HHALF + 2, 2
            else:
                nc.vector.memset(t[:, WR - 2:WR, 2:W + 2], 0.0)
                rlo, rhi, r0 = HHALF - 2, H, 0
            nr = rhi - rlo
            for (p0, n, c0, pc) in src_slices:
                nc.sync.dma_start(t[p0:p0 + pc, r0:r0 + nr, 2:W + 2],
                                  x_d.ap()[n, c0:c0 + pc, rlo:rhi, :])
            return t

        def xr_relu(t):
            nc.vector.tensor_scalar(t[:], t[:], 0.0, None, OP.max)

        def dw_sweep(xr, dwd, y, evac):
            """Tap-outer depthwise over all 4 slices of this half: one
            weight load per tap for 4 matmuls."""
            ps = [dwps.tile([128, HS, W], f32, tag="dwps", name=f"dwps{k}")
                  for k in range(SLH)]
            for t in range(9):
                i, j = t // 3, t % 3
                for k in range(SLH):
                    r = k * HS + 2 * i
                    nc.tensor.matmul(ps[k][:], dwd[:, t, :],
                                     xr[:, r:r + HS, 2 * j:2 * j + W],
                                     start=(t == 0), stop=(t == 8))
            for k in range(SLH):
                ydst = y[:, k * HS * W:(k + 1) * HS * W]
                if evac == "act":
                    nc.scalar.activation(ydst, ps[k][:], AF.Copy)
                else:
                    nc.vector.tensor_copy(ydst, ps[k][:])

        ytiles = {}
        xrtiles = {}

        def emit_xr_dma(p, h):
            na, nb = 2 * p, 2 * p + 1
            xa = xr_dma([(0, na, 0, 128)], h)
            xb = xr_dma([(0, nb, 0, 128)], h)
            xc = xr_dma([(0, na, 128, 64), (64, nb, 128, 64)], h)
            xrtiles[(p, h)] = (xa, xb, xc)

        def dw_sweeps(p, h):
            """Return 3 thunks, one per depthwise 4-slice sweep."""
            xa, xb, xc = xrtiles[(p, h)]
            y0a = ypool.tile([128, HPIX], bf16, tag="y0a")
            y0b = ypool.tile([128, HPIX], bf16, tag="y0b")
            y1p = ypool.tile([128, HPIX], bf16, tag="y1p")
            ytiles[(p, h)] = (y0a, y0b, y1p)
            return [lambda: dw_sweep(xa, dwd0[:], y0a, "dve"),
                    lambda: dw_sweep(xb, dwd0[:], y0b, "act"),
                    lambda: dw_sweep(xc, dwd1[:], y1p, "dve")]

        def pw_step(p, h, s):
            na, nb = 2 * p, 2 * p + 1
            y0a, y0b, y1p = ytiles[(p, h)]
            sl = slice(s * HS * W, (s + 1) * HS * W)
            col = h * SLH + s
            zsl = slice((h * SLH + s) * HS * W, (h * SLH + s + 1) * HS * W)

            # per-bank k0 then k1 back to back: each PSUM bank completes as
            # early as possible so its evacuation can start while the later
            # banks still stream (bufs=1 per tag recycles sooner).
            za = pwps.tile([128, HS * W], f32, tag="pwa")
            zb = pwps.tile([128, HS * W], f32, tag="pwb")
            zca = pwps.tile([64, HS * W], f32, tag="pwc")
            zcb = pwps.tile([64, HS * W], f32, tag="pwd")
            nc.tensor.matmul(za[:], pwlo[:, 0:128], y0a[:, sl],
                             start=True, stop=False)
            nc.tensor.matmul(za[:], pwhi[0:64, 0:128], y1p[0:64, sl],
                             start=False, stop=True)
            nc.tensor.matmul(zb[:], pwlo[:, 0:128], y0b[:, sl],
                             start=True, stop=False)
            nc.tensor.matmul(zb[:], pwhi[64:128, 0:128], y1p[64:128, sl],
                             start=False, stop=True)
            nc.tensor.matmul(zca[:], pwlo[:, 128:192], y0a[:, sl],
                             start=True, stop=False)
            nc.tensor.matmul(zca[:], pwhi[0:64, 128:192], y1p[0:64, sl],
                             start=False, stop=True)
            nc.tensor.matmul(zcb[:], pwlo[:, 128:192], y0b[:, sl],
                             start=True, stop=False)
            nc.tensor.matmul(zcb[:], pwhi[64:128, 128:192], y1p[64:128, sl],
                             start=False, stop=True)

            # evac split across ACT (za, zca) and DVE (zb, zcb)
            dsta = zar0[:, na * PIX:(na + 1) * PIX][:, zsl]
            nc.scalar.activation(dsta, za[:], AF.Copy,
                                 accum_out=sum0[:, na * 8 + col:
                                                na * 8 + col + 1])
            dstb = zar0[:, nb * PIX:(nb + 1) * PIX][:, zsl]
            nc.vector.tensor_scalar(dstb, zb[:], 1.0, None, OP.mult, OP.add,
                                    accum_out=sum0[:, nb * 8 + col:
                                                   nb * 8 + col + 1])
            dstc = zar1[:, na * PIX:(na + 1) * PIX][:, zsl]
            nc.scalar.activation(dstc, zca[:], AF.Copy,
                                 accum_out=sum1[:, na * 8 + col:
                                                na * 8 + col + 1])
            dstd = zar1[:, nb * PIX:(nb + 1) * PIX][:, zsl]
            nc.vector.tensor_scalar(dstd, zcb[:], 1.0, None, OP.mult, OP.add,
                                    accum_out=sum1[:, nb * 8 + col:
                                                   nb * 8 + col + 1])
            # sumsq from the bf16 z copies: za/zca on ACT, zb/zcb on DVE
            for dst, arena, cidx, tg, pc in (
                    (dsta, sq0, na, "sqa", 128), (dstc, sq1, na, "sqc", 64)):
                sqt = sqpool.tile([pc, HS * W], bf16, tag=tg, name="sqt")
                nc.scalar.activation(
                    sqt[:], dst, AF.Square,
                    accum_out=arena[:, cidx * 8 + col:cidx * 8 + col + 1])
            for dst, arena, cidx, tg, pc in (
                    (dstb, sq0, nb, "sqb", 128), (dstd, sq1, nb, "sqd", 64)):
                sqt = sqpool.tile([pc, HS * W], bf16, tag=tg, name="sqt")
                nc.vector.scalar_tensor_tensor(
                    sqt[:], dst, 1.0, dst, OP.mult, OP.mult,
                    accum_out=arena[:, cidx * 8 + col:cidx * 8 + col + 1])

        # stats partial reduce + AllGather (half the steps of AllReduce; the
        # 8-way sum happens locally on DVE after the gather). Part 0 covers
        # images 0..3 and is issued mid-phase-1 so its latency hides under
        # compute; part 1 covers images 4..7 in the tail.
        def emit_ar(part):
            c0, c1 = (0, 32) if part == 0 else (32, 64)
            for arena, row, p0, pc, tg in (
                    (sum0, 0, 0, 128, "s0"), (sq0, 1, 0, 128, "q0"),
                    (sum1, 0, 128, 64, "s1"), (sq1, 1, 128, 64, "q1")):
                r = spool.tile([pc, 1], f32, tag=f"{tg}r{part}",
                               name=f"{tg}r{part}")
                nc.vector.tensor_reduce(r[:], arena[:, c0:c1],
                                        mybir.AxisListType.X, OP.add)
                nc.gpsimd.dma_start(
                    st_l[part].ap()[p0:p0 + pc, row:row + 1], r[:])
            nc.gpsimd.collective_compute(
                "AllGather", OP.bypass,
                replica_groups=[list(range(N_CORES))],
                ins=[st_l[part].ap()], outs=[st_g[part].ap()])

        # software pipeline over 8 (pair, half) cycles. Per cycle kk:
        #   - x DMAs for kk+1 (borders+loads; relu comes later, mid-cycle,
        #     so queued PSUM evacs are not blocked behind the DMA wait)
        #   - dw sweeps of kk interleaved with pw steps of kk-1 (a pw step
        #     between consecutive sweeps gets ~4us of evac slack -> no
        #     PSUM-slot stalls with single-buffered pw psum tags)
        work = [(p, h) for p in range(NPAIR) for h in range(2)]
        emit_xr_dma(*work[0])
        load_rest_of_consts()
        for t in xrtiles[work[0]]:
            xr_relu(t)
        for kk in range(len(work)):
            if kk + 1 < len(work):
                emit_xr_dma(*work[kk + 1])
            sweeps = dw_sweeps(*work[kk])
            pws = ([(lambda s=s: pw_step(*work[kk - 1], s))
                    for s in range(SLH)] if kk > 0 else [None] * SLH)
            sweeps[0]()
            if pws[0]:
                pws[0]()
            sweeps[1]()
            if pws[1]:
                pws[1]()
            if kk + 1 < len(work):
                for t in xrtiles[work[kk + 1]]:
                    xr_relu(t)
            sweeps[2]()
            if pws[2]:
                pws[2]()
            if pws[3]:
                pws[3]()
            if kk == 4:
                emit_ar(0)
        for s in range(SLH):
            pw_step(*work[-1], s)
        emit_ar(1)

        # release phase-1 pools before phase 2
        p1ctx.close()
        p2out = ctx.enter_context(tc.tile_pool(name="p2o", bufs=4))

        # ---- BN coefficients a, b per chunk ----
        def coeffs(gs, gb, pc, tagp):
            mean = spool.tile([pc, 1], f32, tag=f"mean{tagp}")
            nc.vector.tensor_scalar(mean[:], gs[:, 0:1], 1.0 / NTOT, None,
                                    OP.mult)
            ex2 = spool.tile([pc, 1], f32, tag=f"ex2{tagp}")
            nc.vector.tensor_scalar(ex2[:], gs[:, 1:2], 1.0 / NTOT, None,
                                    OP.mult)
            varp = spool.tile([pc, 1], f32, tag=f"varp{tagp}")
            nc.vector.scalar_tensor_tensor(varp[:], mean[:], -1.0, mean[:],
                                           OP.mult, OP.mult)
            nc.vector.tensor_tensor(varp[:], varp[:], ex2[:], OP.add)
            nc.vector.tensor_scalar(varp[:], varp[:], float(BN_EPS), None,
                                    OP.add)
            inv = spool.tile([pc, 1], f32, tag=f"inv{tagp}")
            nc.vector.reciprocal(inv[:], varp[:])
            r0 = spool.tile([pc, 1], f32, tag=f"r0{tagp}")
            nc.scalar.activation(r0[:], inv[:], AF.Sqrt)
            t1 = spool.tile([pc, 1], f32, tag=f"t1{tagp}")
            nc.vector.tensor_tensor(t1[:], r0[:], r0[:], OP.mult)
            nc.vector.scalar_tensor_tensor(t1[:], t1[:], -0.5, varp[:],
                                           OP.mult, OP.mult)
            nc.vector.tensor_scalar(t1[:], t1[:], 1.5, None, OP.add)
            r = spool.tile([pc, 1], f32, tag=f"r{tagp}")
            nc.vector.tensor_tensor(r[:], r0[:], t1[:], OP.mult)
            a = spool.tile([pc, 1], f32, tag=f"a{tagp}")
            nc.vector.tensor_tensor(a[:], r[:], gb[:, 0:1], OP.mult)
            nb_ = spool.tile([pc, 1], f32, tag=f"nb{tagp}")
            nc.vector.scalar_tensor_tensor(nb_[:], mean[:], -1.0, a[:],
                                           OP.mult, OP.mult)
            b = spool.tile([pc, 1], f32, tag=f"b{tagp}")
            nc.vector.tensor_tensor(b[:], gb[:, 1:2], nb_[:], OP.add)
            return a, b

        # gather in the [8 ranks, 2 rows, ch] results per part, sum the rank
        # and part dims locally on DVE
        gparts = []
        for part in range(2):
            gsrc = st_g[part].ap().rearrange("(r c) a -> c a r", r=N_CORES)
            g0 = spool.tile([128, 2, 8], f32, tag=f"g0p{part}",
                            name=f"g0p{part}")
            nc.gpsimd.dma_start(g0[:], gsrc[0:128])
            g1 = spool.tile([64, 2, 8], f32, tag=f"g1p{part}",
                            name=f"g1p{part}")
            nc.gpsimd.dma_start(g1[:], gsrc[128:192])
            gparts.append((g0, g1))
        gs0p = [spool.tile([128, 2], f32, tag=f"gs0p{p}", name=f"gs0p{p}")
                for p in range(2)]
        gs1p = [spool.tile([64, 2], f32, tag=f"gs1p{p}", name=f"gs1p{p}")
                for p in range(2)]
        for p in range(2):
            nc.vector.tensor_reduce(gs0p[p][:], gparts[p][0][:],
                                    mybir.AxisListType.X, OP.add)
            nc.vector.tensor_reduce(gs1p[p][:], gparts[p][1][:],
                                    mybir.AxisListType.X, OP.add)
        gs0 = spool.tile([128, 2], f32, name="gs0")
        nc.vector.tensor_tensor(gs0[:], gs0p[0][:], gs0p[1][:], OP.add)
        gs1 = spool.tile([64, 2], f32, name="gs1")
        nc.vector.tensor_tensor(gs1[:], gs1p[0][:], gs1p[1][:], OP.add)
        a0, b0 = coeffs(gs0, gb0, 128, "0")
        a1, b1 = coeffs(gs1, gb1, 64, "1")

        # ---- phase 2: out = a*z + b (bf16). chunk0 images 0-4 + all of
        # chunk1 on DVE, chunk0 images 5-7 on GpSimd; out DMA split across
        # the sync (chunk0) and scalar (chunk1) queues. ----
        PW2 = 2048
        for n in range(NPER):
            for s in range(PIX // PW2):
                if n < 6:
                    ot = p2out.tile([128, PW2], bf16, tag="ot0")
                    nc.vector.tensor_scalar(
                        ot[:],
                        zar0[:, n * PIX + s * PW2:n * PIX + (s + 1) * PW2],
                        a0[:], b0[:], OP.mult, OP.add)
                    nc.sync.dma_start(
                        out_d.ap()[n, 0:128, :, :].rearrange(
                            "c h w -> c (h w)")[:, s * PW2:(s + 1) * PW2],
                        ot[:])
                else:
                    ot = p2out.tile([128, PW2], bf16, tag="ot0g")
                    nc.gpsimd.tensor_scalar(
                        ot[:],
                        zar0[:, n * PIX + s * PW2:n * PIX + (s + 1) * PW2],
                        a0[:], b0[:], OP.mult, OP.add)
                    nc.gpsimd.dma_start(
                        out_d.ap()[n, 0:128, :, :].rearrange(
                            "c h w -> c (h w)")[:, s * PW2:(s + 1) * PW2],
                        ot[:])
                ot1 = p2out.tile([64, PW2], bf16, tag="ot1")
                nc.vector.tensor_scalar(
                    ot1[:], zar1[:, n * PIX + s * PW2:n * PIX + (s + 1) * PW2],
                    a1[:], b1[:], OP.mult, OP.add)
                nc.scalar.dma_start(
                    out_d.ap()[n, 128:192, :, :].rearrange(
                        "c h w -> c (h w)")[:, s * PW2:(s + 1) * PW2],
                    ot1[:])

    nc.compile()
    return nc


_CACHE = {}


def _get_nc():
    if "nc" not in _CACHE:
        import concourse.bass as bass
        import concourse.tile as tile
        from concourse import mybir
        _CACHE["nc"] = _build(bass, tile, mybir)
    return _CACHE["nc"]


def make_in_maps(x, dw_w, pw_w, gamma, beta):
    """Host-side prep: shard x, build diagonal dw matrices, pwT, gamma/beta."""
    import ml_dtypes
    bf = ml_dtypes.bfloat16
    x = np.ascontiguousarray(np.asarray(x, dtype=np.float32)).astype(bf)
    dw = np.asarray(dw_w, dtype=np.float32).reshape(C, K, K)
    pw = np.asarray(pw_w, dtype=np.float32)
    dwd0 = np.zeros((128, 9, 128), dtype=np.float32)
    dwd1 = np.zeros((128, 9, 128), dtype=np.float32)
    for i in range(3):
        for j in range(3):
            t = i * 3 + j
            for p in range(128):
                dwd0[p, t, p] = dw[p, i, j]
                dwd1[p, t, p] = dw[128 + (p % 64), i, j]
    pwT = np.ascontiguousarray(pw.T)  # [c_in, c_out]
    pwt_lo = pwT[0:128]
    pwt_hi = np.concatenate([pwT[128:192], pwT[128:192]], axis=0)
    g = np.asarray(gamma, np.float32)
    b = np.asarray(beta, np.float32)
    gb0 = np.stack([g[0:128], b[0:128]])
    gb1 = np.stack([g[128:192], b[128:192]])
    in_maps = []
    for c in range(N_CORES):
        in_maps.append({
            "x": x[c * NPER:(c + 1) * NPER],
            "dwd0": dwd0.astype(bf), "dwd1p": dwd1.astype(bf),
            "pwt_lo": np.ascontiguousarray(pwt_lo).astype(bf),
            "pwt_hi": np.ascontiguousarray(pwt_hi).astype(bf),
            "gb0": gb0, "gb1": gb1,
        })
    return in_maps


def kernel(x, dw_w, pw_w, gamma, beta, trace=False, tmpdir=None):
    from concourse.bass_utils import run_bass_kernel_spmd
    nc = _get_nc()
    in_maps = make_in_maps(x, dw_w, pw_w, gamma, beta)
    res = run_bass_kernel_spmd(nc, in_maps, core_ids=list(range(N_CORES)),
                               trace=trace, tmpdir=tmpdir)
    out = np.concatenate(
        [np.asarray(res.results[c]["out"]).astype(np.float32)
         for c in range(N_CORES)], axis=0)
    if trace:
        _CACHE["last_result"] = res
    return out


# revision 53
# speedup vs baseline: 1.0277x; 1.0277x over previous
"""Trainium2 Bass kernel for nn_DilConv: relu -> 3x3 depthwise dilated conv
(dilation=2, pad=2) -> 1x1 pointwise conv (192->192) -> BatchNorm (training
mode, global batch stats) on x[64,192,64,64] f32.

Sharding: data-parallel over batch N across 8 cores (8 images/core).
Sync-BN via an AllReduce of per-channel (sum, sumsq) of z.

v2 design (vs f32r baseline):
- bf16 matmul path end to end: f32r lowers to FP32_HIGH mode (~2 cycles/row,
  no fast weight load); bf16 streams 1 cycle/row with FWL. x is cast to bf16
  on the host, DMA'd into padded window tiles, relu'd in place on DVE.
- Depthwise loops are tap-outer over 2-slice groups so consecutive matmuls
  share lhsT (weight-load reuse) with PSUM at 4 banks double-buffered.
- Channel chunk1 (64 ch) of two images is paired on 128 partitions: one
  diagonal matmul computes both images' depthwise output (25% fewer PE
  rows). Pointwise consumes the paired y via base-64 row tiles (duplicated
  pwT_hi weights). All tile positions are row-aligned; no column tiling.
- z stays in SBUF as bf16 arenas - no DRAM roundtrip. Output is written
  bf16 and upcast on the host (rel tol 2e-2 allows it).
- dw sweeps of pair k interleave with pw steps of pair k-1 so the PE stream
  is dense while PSUM evacuations (split across ACT and DVE) keep pace.
"""

import sys

import numpy as np

sys.path.insert(0, "/opt/trn_rl_repo")

N_CORES = 8
N, C, H, W = 64, 192, 64, 64
NPER = N // N_CORES  # images per core
NPAIR = NPER // 2
K, DIL, PAD = 3, 2, 2
BN_EPS = 1e-5
HS = 8           # h rows per psum slice (8*64 = 512 = fp32 psum bank)
HHALF = 32       # rows per half-image working set
SLH = HHALF // HS  # 4 slices per half
WR = 36          # window rows per half (32 + 2*2 halo)
WC = W + 4       # 68 window cols
PIX = H * W      # 4096 pixels/image
HPIX = HHALF * W  # 2048 pixels per half
NTOT = float(N * PIX)  # global BN count


def _build(nc_mod, tile_mod, mybir):
    from contextlib import ExitStack

    f32 = mybir.dt.float32
    bf16 = mybir.dt.bfloat16
    AF = mybir.ActivationFunctionType
    OP = mybir.AluOpType

    import concourse.bacc as bacc

    nc = bacc.Bacc("TRN2", target_bir_lowering=False, debug=False,
                   num_devices=N_CORES)

    x_d = nc.dram_tensor("x", [NPER, C, H, W], bf16, kind="ExternalInput")
    dwd0_d = nc.dram_tensor("dwd0", [128, 9, 128], bf16, kind="ExternalInput")
    dwd1_d = nc.dram_tensor("dwd1p", [128, 9, 128], bf16, kind="ExternalInput")
    pwlo_d = nc.dram_tensor("pwt_lo", [128, 192], bf16, kind="ExternalInput")
    pwhi_d = nc.dram_tensor("pwt_hi", [128, 192], bf16, kind="ExternalInput")
    gb0_d = nc.dram_tensor("gb0", [2, 128], f32, kind="ExternalInput")
    gb1_d = nc.dram_tensor("gb1", [2, 64], f32, kind="ExternalInput")
    out_d = nc.dram_tensor("out", [NPER, C, H, W], bf16, kind="ExternalOutput")
    st_l = [nc.dram_tensor("stats_l0", [192, 2], f32, kind="Internal")]
    st_g = [nc.dram_tensor("stats_g0", [N_CORES * 192, 2], f32,
                           kind="Internal", addr_space="Shared")]

    with tile_mod.TileContext(nc) as tc, ExitStack() as ctx:
        const = ctx.enter_context(tc.tile_pool(name="const", bufs=1))
        spool = ctx.enter_context(tc.tile_pool(name="stats", bufs=1))
        zpool = ctx.enter_context(tc.tile_pool(name="zarena", bufs=1))
        p1ctx = ctx.enter_context(ExitStack())
        xrpool = p1ctx.enter_context(tc.tile_pool(name="xr", bufs=6))
        ypool = p1ctx.enter_context(tc.tile_pool(name="y", bufs=2))
        sqpool = p1ctx.enter_context(tc.tile_pool(name="sq", bufs=1))
        dwps = p1ctx.enter_context(tc.tile_pool(name="dwps", bufs=4,
                                                space="PSUM"))
        pwps = p1ctx.enter_context(tc.tile_pool(name="pwps", bufs=1,
                                                space="PSUM"))

        # ---- constants (dwd0 first: the first dw sweep needs only it) ----
        dwd0 = const.tile([128, 9, 128], bf16)
        nc.sync.dma_start(dwd0[:], dwd0_d.ap())
        dwd1 = const.tile([128, 9, 128], bf16)
        pwlo = const.tile([128, 192], bf16)
        pwhi = const.tile([128, 192], bf16)
        gb0 = const.tile([128, 2], f32)
        gb1 = const.tile([64, 2], f32)

        def load_rest_of_consts():
            nc.sync.dma_start(dwd1[:], dwd1_d.ap())
            nc.sync.dma_start(pwlo[:], pwlo_d.ap())
            nc.sync.dma_start(pwhi[:], pwhi_d.ap())
            nc.sync.dma_start(gb0[:], gb0_d.ap().rearrange("a c -> c a"))
            nc.sync.dma_start(gb1[:], gb1_d.ap().rearrange("a c -> c a"))



        # z arenas in SBUF (bf16)
        zar0 = zpool.tile([128, NPER * PIX], bf16, name="zar0")
        zar1 = zpool.tile([64, NPER * PIX], bf16, name="zar1")

        # stats arenas: one column per pw evac instruction
        sum0 = spool.tile([128, NPER * 8], f32, name="sum0")
        sq0 = spool.tile([128, NPER * 8], f32, name="sq0")
        sum1 = spool.tile([64, NPER * 8], f32, name="sum1")
        sq1 = spool.tile([64, NPER * 8], f32, name="sq1")

        # ---- phase 1 ----
        def xr_dma(src_slices, h):
            """Zero borders + DMA x into a window tile [128, WR, WC] bf16.
            The relu pass is emitted separately (xr_relu) so PSUM evacs
            queued on DVE are not head-of-line blocked behind the x DMA."""
            t = xrpool.tile([128, WR, WC], bf16, tag="xr")
            nc.vector.memset(t[:, :, 0:2], 0.0)
            nc.vector.memset(t[:, :, W + 2:W + 4], 0.0)
            if h == 0:
                nc.vector.memset(t[:, 0:2, 2:W + 2], 0.0)
                rlo, rhi, r0 = 0, HHALF + 2, 2
            else:
                nc.vector.memset(t[:, WR - 2:WR, 2:W + 2], 0.0)
                rlo, rhi, r0 = HHALF - 2, H, 0
            nr = rhi - rlo
            for (p0, n, c0, pc) in src_slices:
                nc.sync.dma_start(t[p0:p0 + pc, r0:r0 + nr, 2:W + 2],
                                  x_d.ap()[n, c0:c0 + pc, rlo:rhi, :])
            return t

        def xr_relu(t):
            nc.vector.tensor_scalar(t[:], t[:], 0.0, None, OP.max)

        def dw_sweep(xr, dwd, y, s0, evac):
            """Tap-outer depthwise over slices s0, s0+1 of this half."""
            ps = [dwps.tile([128, HS, W], f32, tag="dwps", name=f"dwps{k}")
                  for k in range(2)]
            for t in range(9):
                i, j = t // 3, t % 3
                for k in range(2):
                    r = (s0 + k) * HS + 2 * i
                    nc.tensor.matmul(ps[k][:], dwd[:, t, :],
                                     xr[:, r:r + HS, 2 * j:2 * j + W],
                                     start=(t == 0), stop=(t == 8))
            for k in range(2):
                ydst = y[:, (s0 + k) * HS * W:(s0 + k + 1) * HS * W]
                if evac == "act":
                    nc.scalar.activation(ydst, ps[k][:], AF.Copy)
                else:
                    nc.vector.tensor_copy(ydst, ps[k][:])

        ytiles = {}
        xrtiles = {}

        def emit_xr_dma(p, h):
            na, nb = 2 * p, 2 * p + 1
            xa = xr_dma([(0, na, 0, 128)], h)
            xb = xr_dma([(0, nb, 0, 128)], h)
            xc = xr_dma([(0, na, 128, 64), (64, nb, 128, 64)], h)
            xrtiles[(p, h)] = (xa, xb, xc)

        def dw_sweeps(p, h):
            """Return 6 thunks, one per depthwise 2-slice sweep."""
            xa, xb, xc = xrtiles[(p, h)]
            y0a = ypool.tile([128, HPIX], bf16, tag="y0a")
            y0b = ypool.tile([128, HPIX], bf16, tag="y0b")
            y1p = ypool.tile([128, HPIX], bf16, tag="y1p")
            ytiles[(p, h)] = (y0a, y0b, y1p)
            out = []
            for s0 in (0, 2):
                out.append(lambda s0=s0: dw_sweep(xa, dwd0[:], y0a, s0, "dve"))
            for s0 in (0, 2):
                out.append(lambda s0=s0: dw_sweep(xb, dwd0[:], y0b, s0, "act"))
            for s0 in (0, 2):
                out.append(lambda s0=s0: dw_sweep(xc, dwd1[:], y1p, s0, "dve"))
            return out

        def pw_step(p, h, s):
            na, nb = 2 * p, 2 * p + 1
            y0a, y0b, y1p = ytiles[(p, h)]
            sl = slice(s * HS * W, (s + 1) * HS * W)
            col = h * SLH + s
            zsl = slice((h * SLH + s) * HS * W, (h * SLH + s + 1) * HS * W)

            # per-bank k0 then k1 back to back: each PSUM bank completes as
            # early as possible so its evacuation can start while the later
            # banks still stream (bufs=1 per tag recycles sooner).
            za = pwps.tile([128, HS * W], f32, tag="pwa")
            zb = pwps.tile([128, HS * W], f32, tag="pwb")
            zca = pwps.tile([64, HS * W], f32, tag="pwc")
            zcb = pwps.tile([64, HS * W], f32, tag="pwd")
            nc.tensor.matmul(za[:], pwlo[:, 0:128], y0a[:, sl],
                             start=True, stop=False)
            nc.tensor.matmul(za[:], pwhi[0:64, 0:128], y1p[0:64, sl],
                             start=False, stop=True)
            nc.tensor.matmul(zb[:], pwlo[:, 0:128], y0b[:, sl],
                             start=True, stop=False)
            nc.tensor.matmul(zb[:], pwhi[64:128, 0:128], y1p[64:128, sl],
                             start=False, stop=True)
            nc.tensor.matmul(zca[:], pwlo[:, 128:192], y0a[:, sl],
                             start=True, stop=False)
            nc.tensor.matmul(zca[:], pwhi[0:64, 128:192], y1p[0:64, sl],
                             start=False, stop=True)
            nc.tensor.matmul(zcb[:], pwlo[:, 128:192], y0b[:, sl],
                             start=True, stop=False)
            nc.tensor.matmul(zcb[:], pwhi[64:128, 128:192], y1p[64:128, sl],
                             start=False, stop=True)

            # evac split across ACT (za, zca) and DVE (zb, zcb)
            dsta = zar0[:, na * PIX:(na + 1) * PIX][:, zsl]
            nc.scalar.activation(dsta, za[:], AF.Copy,
                                 accum_out=sum0[:, na * 8 + col:
                                                na * 8 + col + 1])
            dstb = zar0[:, nb * PIX:(nb + 1) * PIX][:, zsl]
            nc.vector.tensor_scalar(dstb, zb[:], 1.0, None, OP.mult, OP.add,
                                    accum_out=sum0[:, nb * 8 + col:
                                                   nb * 8 + col + 1])
            dstc = zar1[:, na * PIX:(na + 1) * PIX][:, zsl]
            nc.scalar.activation(dstc, zca[:], AF.Copy,
                                 accum_out=sum1[:, na * 8 + col:
                                                na * 8 + col + 1])
            dstd = zar1[:, nb * PIX:(nb + 1) * PIX][:, zsl]
            nc.vector.tensor_scalar(dstd, zcb[:], 1.0, None, OP.mult, OP.add,
                                    accum_out=sum1[:, nb * 8 + col:
                                                   nb * 8 + col + 1])
            # sumsq from the bf16 z copies: za/zca on ACT, zb/zcb on DVE
            for dst, arena, cidx, tg, pc in (
                    (dsta, sq0, na, "sqa", 128), (dstc, sq1, na, "sqc", 64)):
                sqt = sqpool.tile([pc, HS * W], bf16, tag=tg, name="sqt")
                nc.scalar.activation(
                    sqt[:], dst, AF.Square,
                    accum_out=arena[:, cidx * 8 + col:cidx * 8 + col + 1])
            for dst, arena, cidx, tg, pc in (
                    (dstb, sq0, nb, "sqb", 128), (dstd, sq1, nb, "sqd", 64)):
                sqt = sqpool.tile([pc, HS * W], bf16, tag=tg, name="sqt")
                nc.vector.scalar_tensor_tensor(
                    sqt[:], dst, 1.0, dst, OP.mult, OP.mult,
                    accum_out=arena[:, cidx * 8 + col:cidx * 8 + col + 1])

        # stats reduce + one tail AllGather (half the steps of AllReduce;
        # the 8-way sum happens locally on DVE after the gather). A split
        # "hidden" collective mid-phase-1 was tried and reliably cost ~24us
        # of PE stall in its window, more than it saved in the tail.
        def emit_ar(part):
            c0, c1 = 0, 64
            for arena, row, p0, pc, tg in (
                    (sum0, 0, 0, 128, "s0"), (sq0, 1, 0, 128, "q0"),
                    (sum1, 0, 128, 64, "s1"), (sq1, 1, 128, 64, "q1")):
                r = spool.tile([pc, 1], f32, tag=f"{tg}r{part}",
                               name=f"{tg}r{part}")
                nc.vector.tensor_reduce(r[:], arena[:, c0:c1],
                                        mybir.AxisListType.X, OP.add)
                nc.gpsimd.dma_start(
                    st_l[part].ap()[p0:p0 + pc, row:row + 1], r[:])
            nc.gpsimd.collective_compute(
                "AllGather", OP.bypass,
                replica_groups=[list(range(N_CORES))],
                ins=[st_l[part].ap()], outs=[st_g[part].ap()])

        # software pipeline over 8 (pair, half) cycles. Per cycle kk:
        #   - x DMAs for kk+1 (borders+loads; relu comes later, mid-cycle,
        #     so queued PSUM evacs are not blocked behind the DMA wait)
        #   - dw sweeps of kk interleaved with pw steps of kk-1 (a pw step
        #     between consecutive sweeps gets ~4us of evac slack -> no
        #     PSUM-slot stalls with single-buffered pw psum tags)
        work = [(p, h) for p in range(NPAIR) for h in range(2)]
        emit_xr_dma(*work[0])
        load_rest_of_consts()
        for t in xrtiles[work[0]]:
            xr_relu(t)
        for kk in range(len(work)):
            if kk + 1 < len(work):
                emit_xr_dma(*work[kk + 1])
            sweeps = dw_sweeps(*work[kk])
            pws = ([(lambda s=s: pw_step(*work[kk - 1], s))
                    for s in range(SLH)] if kk > 0 else [None] * SLH)
            sweeps[0]()
            if pws[0]:
                pws[0]()
            sweeps[1]()
            if pws[1]:
                pws[1]()
            if kk + 1 < len(work):
                for t in xrtiles[work[kk + 1]]:
                    xr_relu(t)
            sweeps[2]()
            if pws[2]:
                pws[2]()
            sweeps[3]()
            if pws[3]:
                pws[3]()
            sweeps[4]()
            sweeps[5]()
        for s in range(SLH):
            pw_step(*work[-1], s)
        emit_ar(0)

        # release phase-1 pools before phase 2
        p1ctx.close()
        p2out = ctx.enter_context(tc.tile_pool(name="p2o", bufs=4))

        # ---- BN coefficients a, b per chunk ----
        def coeffs(gs, gb, pc, tagp):
            mean = spool.tile([pc, 1], f32, tag=f"mean{tagp}")
            nc.vector.tensor_scalar(mean[:], gs[:, 0:1], 1.0 / NTOT, None,
                                    OP.mult)
            ex2 = spool.tile([pc, 1], f32, tag=f"ex2{tagp}")
            nc.vector.tensor_scalar(ex2[:], gs[:, 1:2], 1.0 / NTOT, None,
                                    OP.mult)
            varp = spool.tile([pc, 1], f32, tag=f"varp{tagp}")
            nc.vector.scalar_tensor_tensor(varp[:], mean[:], -1.0, mean[:],
                                           OP.mult, OP.mult)
            nc.vector.tensor_tensor(varp[:], varp[:], ex2[:], OP.add)
            nc.vector.tensor_scalar(varp[:], varp[:], float(BN_EPS), None,
                                    OP.add)
            inv = spool.tile([pc, 1], f32, tag=f"inv{tagp}")
            nc.vector.reciprocal(inv[:], varp[:])
            r0 = spool.tile([pc, 1], f32, tag=f"r0{tagp}")
            nc.scalar.activation(r0[:], inv[:], AF.Sqrt)
            t1 = spool.tile([pc, 1], f32, tag=f"t1{tagp}")
            nc.vector.tensor_tensor(t1[:], r0[:], r0[:], OP.mult)
            nc.vector.scalar_tensor_tensor(t1[:], t1[:], -0.5, varp[:],
                                           OP.mult, OP.mult)
            nc.vector.tensor_scalar(t1[:], t1[:], 1.5, None, OP.add)
            r = spool.tile([pc, 1], f32, tag=f"r{tagp}")
            nc.vector.tensor_tensor(r[:], r0[:], t1[:], OP.mult)
            a = spool.tile([pc, 1], f32, tag=f"a{tagp}")
            nc.vector.tensor_tensor(a[:], r[:], gb[:, 0:1], OP.mult)
            nb_ = spool.tile([pc, 1], f32, tag=f"nb{tagp}")
            nc.vector.scalar_tensor_tensor(nb_[:], mean[:], -1.0, a[:],
                                           OP.mult, OP.mult)
            b = spool.tile([pc, 1], f32, tag=f"b{tagp}")
            nc.vector.tensor_tensor(b[:], gb[:, 1:2], nb_[:], OP.add)
            return a, b

        # gather in the [8 ranks, ch, 2] result, sum the rank dim on DVE
        gsrc = st_g[0].ap().rearrange("(r c) a -> c a r", r=N_CORES)
        g0 = spool.tile([128, 2, 8], f32, name="g0")
        nc.gpsimd.dma_start(g0[:], gsrc[0:128])
        g1 = spool.tile([64, 2, 8], f32, name="g1")
        nc.gpsimd.dma_start(g1[:], gsrc[128:192])
        gs0 = spool.tile([128, 2], f32, name="gs0")
        nc.vector.tensor_reduce(gs0[:], g0[:], mybir.AxisListType.X, OP.add)
        gs1 = spool.tile([64, 2], f32, name="gs1")
        nc.vector.tensor_reduce(gs1[:], g1[:], mybir.AxisListType.X, OP.add)
        a0, b0 = coeffs(gs0, gb0, 128, "0")
        a1, b1 = coeffs(gs1, gb1, 64, "1")

        # ---- phase 2: out = a*z + b (bf16). chunk0 images 0-4 + all of
        # chunk1 on DVE, chunk0 images 5-7 on GpSimd; out DMA split across
        # the sync (chunk0) and scalar (chunk1) queues. ----
        PW2 = 2048
        for n in range(NPER):
            for s in range(PIX // PW2):
                if n < 6:
                    ot = p2out.tile([128, PW2], bf16, tag="ot0")
                    nc.vector.tensor_scalar(
                        ot[:],
                        zar0[:, n * PIX + s * PW2:n * PIX + (s + 1) * PW2],
                        a0[:], b0[:], OP.mult, OP.add)
                    nc.sync.dma_start(
                        out_d.ap()[n, 0:128, :, :].rearrange(
                            "c h w -> c (h w)")[:, s * PW2:(s + 1) * PW2],
                        ot[:])
                else:
                    ot = p2out.tile([128, PW2], bf16, tag="ot0g")
                    nc.gpsimd.tensor_scalar(
                        ot[:],
                        zar0[:, n * PIX + s * PW2:n * PIX + (s + 1) * PW2],
                        a0[:], b0[:], OP.mult, OP.add)
                    nc.gpsimd.dma_start(
                        out_d.ap()[n, 0:128, :, :].rearrange(
                            "c h w -> c (h w)")[:, s * PW2:(s + 1) * PW2],
                        ot[:])
                ot1 = p2out.tile([64, PW2], bf16, tag="ot1")
                nc.vector.tensor_scalar(
                    ot1[:], zar1[:, n * PIX + s * PW2:n * PIX + (s + 1) * PW2],
                    a1[:], b1[:], OP.mult, OP.add)
                nc.scalar.dma_start(
                    out_d.ap()[n, 128:192, :, :].rearrange(
                        "c h w -> c (h w)")[:, s * PW2:(s + 1) * PW2],
                    ot1[:])

    nc.compile()
    return nc


_CACHE = {}


def _get_nc():
    if "nc" not in _CACHE:
        import concourse.bass as bass
        import concourse.tile as tile
        from concourse import mybir
        _CACHE["nc"] = _build(bass, tile, mybir)
    return _CACHE["nc"]


def make_in_maps(x, dw_w, pw_w, gamma, beta):
    """Host-side prep: shard x, build diagonal dw matrices, pwT, gamma/beta."""
    import ml_dtypes
    bf = ml_dtypes.bfloat16
    x = np.ascontiguousarray(np.asarray(x, dtype=np.float32)).astype(bf)
    dw = np.asarray(dw_w, dtype=np.float32).reshape(C, K, K)
    pw = np.asarray(pw_w, dtype=np.float32)
    dwd0 = np.zeros((128, 9, 128), dtype=np.float32)
    dwd1 = np.zeros((128, 9, 128), dtype=np.float32)
    for i in range(3):
        for j in range(3):
            t = i * 3 + j
            for p in range(128):
                dwd0[p, t, p] = dw[p, i, j]
                dwd1[p, t, p] = dw[128 + (p % 64), i, j]
    pwT = np.ascontiguousarray(pw.T)  # [c_in, c_out]
    pwt_lo = pwT[0:128]
    pwt_hi = np.concatenate([pwT[128:192], pwT[128:192]], axis=0)
    g = np.asarray(gamma, np.float32)
    b = np.asarray(beta, np.float32)
    gb0 = np.stack([g[0:128], b[0:128]])
    gb1 = np.stack([g[128:192], b[128:192]])
    in_maps = []
    for c in range(N_CORES):
        in_maps.append({
            "x": x[c * NPER:(c + 1) * NPER],
            "dwd0": dwd0.astype(bf), "dwd1p": dwd1.astype(bf),
            "pwt_lo": np.ascontiguousarray(pwt_lo).astype(bf),
            "pwt_hi": np.ascontiguousarray(pwt_hi).astype(bf),
            "gb0": gb0, "gb1": gb1,
        })
    return in_maps


def kernel(x, dw_w, pw_w, gamma, beta, trace=False, tmpdir=None):
    from concourse.bass_utils import run_bass_kernel_spmd
    nc = _get_nc()
    in_maps = make_in_maps(x, dw_w, pw_w, gamma, beta)
    res = run_bass_kernel_spmd(nc, in_maps, core_ids=list(range(N_CORES)),
                               trace=trace, tmpdir=tmpdir)
    out = np.concatenate(
        [np.asarray(res.results[c]["out"]).astype(np.float32)
         for c in range(N_CORES)], axis=0)
    if trace:
        _CACHE["last_result"] = res
    return out
